# revision 16
# baseline (speedup 1.0000x reference)
"""GPT block (LN -> causal MHA -> residual -> LN -> MLP -> residual) on 8 trn2 cores.

Sharding: core c = (batch b = c//2, parity o = c%2). Each core owns the
interleaved tokens o::2 of its batch. K/V are computed redundantly by the two
cores of a batch; attention rows and the MLP are token-parallel. No cross-core
communication: outputs are scattered back on the host.

v2 structure (keeps the PE dense so the HAM clock gate stays at 8/8):
 - Phase 1 interleaves LN1 (DVE) with the QKV projections (PE) in 512-token
   windows; all pairs' K^T / Q^T / V-augmented tiles stay resident in SBUF.
 - Phase 2 runs attention head-by-head: per key-block j, scores -> exp -> AV
   accumulate into a persistent [65, 1024] PSUM tile (extra ones-row gives the
   softmax denominator). Scores of block j+1 overlap the exp/AV of block j.
 - Phase 3 is the MLP, token-parallel, unchanged in shape.

Matmuls run in bf16 (fp32 PSUM), optionally fp8 e4m3 with DoubleRow
(2x contraction/cycle) for the projections and the MLP; layernorm/softmax
stats stay fp32.
"""

import sys

if "/opt/trn_rl_repo" not in sys.path:
    sys.path.insert(0, "/opt/trn_rl_repo")

import numpy as np
import ml_dtypes

import concourse.bass as bass
import concourse.tile as tile
from concourse import mybir
from concourse.bass_utils import run_bass_kernel_spmd
from concourse.masks import make_identity

B, T, D, H, HD = 4, 2048, 1024, 16, 64
FF = 4 * D
P = 128
NB = T // P        # 16 key blocks
TQ = T // 2        # 1024 query tokens per core
NQ = TQ // P       # 8 query blocks per core
NCH = D // P       # 8 contraction chunks over D
EPS = 1e-5
F32 = mybir.dt.float32
BF16 = mybir.dt.bfloat16
FP8 = mybir.dt.float8e4
DR = mybir.MatmulPerfMode.DoubleRow

FP8_PROJ = False   # QKV projections in fp8 DoubleRow
FP8_MLP = False    # MLP matmuls in fp8 DoubleRow
WS = 32.0          # fp8 weight scale (proj)
MS = 32.0          # fp8 weight scale (mlp)
NEG = -240.0       # exp(0.125*(s-240)) ~ 1e-12: exact-enough zero


def _mm_chain(nc, ps, w_ap, x_ap, nch, fp8):
    """Accumulating matmul chain over `nch` 128-chunks of the contraction.
    w_ap(c, k) / x_ap(c, k) return APs for chunk slice [c, c+k)."""
    if fp8:
        for c in range(0, nch, 2):
            nc.tensor.matmul(ps, lhsT=w_ap(c, 2), rhs=x_ap(c, 2),
                             start=(c == 0), stop=(c == nch - 2), perf_mode=DR)
    else:
        for c in range(nch):
            nc.tensor.matmul(ps, lhsT=w_ap(c, 1), rhs=x_ap(c, 1),
                             start=(c == 0), stop=(c == nch - 1))


def build_program(apply_g1=False, apply_g2=False):
    nc = bass.Bass()
    xdt = FP8 if FP8_PROJ else BF16
    mdt = FP8 if FP8_MLP else BF16
    sc_exp = 0.125 / (WS * WS if FP8_PROJ else 1.0)
    aug = WS if FP8_PROJ else 1.0

    xb = nc.declare_dram_parameter("xb", [T, D], F32, isOutput=False)
    xq = nc.declare_dram_parameter("xq", [TQ, D], F32, isOutput=False)
    wq = nc.declare_dram_parameter("wq", [8, P, NCH, P], xdt, isOutput=False)
    wk = nc.declare_dram_parameter("wk", [8, P, NCH, P], xdt, isOutput=False)
    wv = nc.declare_dram_parameter("wv", [8, P, NCH, P], xdt, isOutput=False)
    w1t = nc.declare_dram_parameter("w1t", [32, P, NCH, P], mdt, isOutput=False)
    w2t = nc.declare_dram_parameter("w2t", [8, P, 32, P], mdt, isOutput=False)
    b1t = nc.declare_dram_parameter("b1t", [P, 32], F32, isOutput=False)
    b2t = nc.declare_dram_parameter("b2t", [P, 8], F32, isOutput=False)
    maskt = nc.declare_dram_parameter("maskt", [P, 64], F32, isOutput=False)
    gb = {}
    if apply_g1:
        gb["g1"] = nc.declare_dram_parameter("g1v", [D], F32, isOutput=False)
        gb["be1"] = nc.declare_dram_parameter("be1v", [D], F32, isOutput=False)
    if apply_g2:
        gb["g2"] = nc.declare_dram_parameter("g2v", [D], F32, isOutput=False)
        gb["be2"] = nc.declare_dram_parameter("be2v", [D], F32, isOutput=False)
    out_d = nc.declare_dram_parameter("out", [TQ, D], F32, isOutput=True)

    Exp = mybir.ActivationFunctionType.Exp
    Relu = mybir.ActivationFunctionType.Relu

    with tile.TileContext(nc) as tc:
        with tc.tile_pool(name="consts", bufs=1) as consts, \
             tc.tile_pool(name="big", bufs=1) as big:
            id_f32 = consts.tile([P, P], F32)
            make_identity(nc, id_f32)
            id_bf = consts.tile([P, P], BF16)
            make_identity(nc, id_bf)
            eps_sb = consts.tile([P, 1], F32)
            nc.vector.memset(eps_sb, EPS)
            mask_sb = consts.tile([P, 64], F32)
            nc.sync.dma_start(out=mask_sb, in_=maskt[:, :])
            b1_sb = consts.tile([P, 32], F32)
            nc.sync.dma_start(out=b1_sb, in_=b1t[:, :])
            b2_sb = consts.tile([P, 8], F32)
            nc.sync.dma_start(out=b2_sb, in_=b2t[:, :])
            inv_sb = consts.tile([P, 1], F32)
            nc.vector.memset(inv_sb, 1.0 / (MS * MS))

            def bcast(name):
                t = consts.tile([P, D], F32, tag=f"bc_{name}")
                src = gb[name]
                ap = bass.AP(tensor=src.tensor if hasattr(src, "tensor") else src[:].tensor,
                             offset=src[:].offset, ap=[[0, P]] + list(src[:].ap))
                nc.sync.dma_start(out=t, in_=ap)
                return t

            g1_t = bcast("g1") if apply_g1 else None
            be1_t = bcast("be1") if apply_g1 else None
            g2_t = bcast("g2") if apply_g2 else None
            be2_t = bcast("be2") if apply_g2 else None

            # ---- resident tensors (phases 1-3) ----
            KT = big.tile([P, 8, T], BF16)             # per pair: [2 heads x 64, keys]
            QT = big.tile([P, 8, TQ], BF16)
            VA = big.tile([P, 16, NB, 65], BF16)       # (pair*2+hh, key block, hd+aug)
            nc.vector.memset(VA[:, :, :, 64:65], aug)
            ACC = big.tile([P, NQ, D], BF16)           # attention out accumulator

            def layernorm_rows(lnp, src_ap, gt, bt):
                """LN over rows of src_ap [128, D] f32 -> xn [128, D] f32."""
                stats = lnp.tile([P, 2, 6], F32, tag="stats")
                for s in range(2):
                    nc.vector.bn_stats(out=stats[:, s, :], in_=src_ap[:, s * 512:(s + 1) * 512])
                mv = lnp.tile([P, 2], F32, tag="mv")
                nc.vector.bn_aggr(out=mv, in_=stats)
                rstd = lnp.tile([P, 1], F32, tag="rstd")
                nc.scalar.activation(out=rstd, in_=mv[:, 1:2],
                                     func=mybir.ActivationFunctionType.Sqrt,
                                     bias=eps_sb, scale=1.0)
                nc.vector.reciprocal(out=rstd, in_=rstd)
                xn = lnp.tile([P, D], F32, tag="xn")
                nc.vector.tensor_scalar(out=xn, in0=src_ap, scalar1=mv[:, 0:1],
                                        scalar2=rstd, op0=mybir.AluOpType.subtract,
                                        op1=mybir.AluOpType.mult)
                if gt is not None:
                    nc.vector.tensor_mul(xn, xn, gt)
                if bt is not None:
                    nc.vector.tensor_add(xn, xn, bt)
                return xn

            def transpose_to(pst, xn, dst3, col, dtype):
                """xn [128, D] -> transposed bf16/fp8 into dst3[:, c, col:col+128]."""
                for half in range(2):
                    ps = pst.tile([P, 4, P], F32, tag="lntr")
                    for cc in range(4):
                        c = half * 4 + cc
                        nc.tensor.matmul(ps[:, cc, :], lhsT=xn[:, c * P:(c + 1) * P],
                                         rhs=id_f32, is_transpose=True,
                                         start=(cc == 0), stop=(cc == 3),
                                         skip_group_check=True)
                    nc.vector.tensor_copy(dst3[:, half * 4:(half + 1) * 4, col:col + P], ps)

            # ---- Phase 1: LN1 + QKV projections, 512-token windows ----
            with tc.tile_pool(name="wpool", bufs=1) as wpool, \
                 tc.tile_pool(name="xtw", bufs=2) as xtwp, \
                 tc.tile_pool(name="xqw", bufs=2) as xqwp, \
                 tc.tile_pool(name="lnsrc", bufs=2) as lnsrc, \
                 tc.tile_pool(name="lnp", bufs=2) as lnp, \
                 tc.tile_pool(name="scr", bufs=3) as scr, \
                 tc.tile_pool(name="pst", bufs=2, space="PSUM") as pst, \
                 tc.tile_pool(name="mmp", bufs=3, space="PSUM") as mmp, \
                 tc.tile_pool(name="tr65", bufs=2, space="PSUM") as tr65:
                wq_s = wpool.tile([P, 8, NCH, P], xdt, tag="wq")
                wk_s = wpool.tile([P, 8, NCH, P], xdt, tag="wk")
                wv_s = wpool.tile([P, 8, NCH, P], xdt, tag="wv")
                for pr in range(8):
                    nc.sync.dma_start(out=wq_s[:, pr], in_=wq[pr])
                    nc.sync.dma_start(out=wk_s[:, pr], in_=wk[pr])
                    nc.sync.dma_start(out=wv_s[:, pr], in_=wv[pr])
                XQw = None
                for tg in range(4):
                    XTw = xtwp.tile([P, NCH, 512], xdt, tag="xtw")
                    if tg % 2 == 0:
                        XQw = xqwp.tile([P, NCH, 512], xdt, tag="xqw")
                    for bi in range(4):
                        blk = tg * 4 + bi
                        x_t = lnsrc.tile([P, D], F32, tag="xsrc")
                        nc.sync.dma_start(out=x_t, in_=xb[blk * P:(blk + 1) * P, :])
                        xn = layernorm_rows(lnp, x_t, g1_t, be1_t)
                        transpose_to(pst, xn, XTw, bi * P, xdt)
                    for qi in range(2):
                        kb = tg * 2 + qi
                        xq_t = lnsrc.tile([P, D], F32, tag="xsrc")
                        nc.sync.dma_start(out=xq_t, in_=xq[kb * P:(kb + 1) * P, :])
                        xn = layernorm_rows(lnp, xq_t, g1_t, be1_t)
                        transpose_to(pst, xn, XQw, (kb % 4) * P, xdt)

                    # K/V projections for this window, all pairs.
                    # V transposes lag one pair so the PE never waits on the
                    # DVE copy of vt.
                    def emit_vtrans(vt, pr):
                        for hh in range(2):
                            tps = tr65.tile([P, 4, 66], BF16, tag="vtr")
                            for s in range(4):
                                nc.tensor.matmul(
                                    tps[:, s, 0:64],
                                    lhsT=vt[hh * 64:(hh + 1) * 64, s * P:(s + 1) * P],
                                    rhs=id_bf[hh * 64:hh * 64 + 64, hh * 64:hh * 64 + 64],
                                    is_transpose=True, start=(s == 0), stop=(s == 3),
                                    skip_group_check=True)
                            nc.vector.tensor_copy(
                                VA[:, pr * 2 + hh, tg * 4:(tg + 1) * 4, 0:64],
                                tps[:, :, 0:64])

                    pending_vt = None
                    for pr in range(8):
                        ps = mmp.tile([P, 512], F32, tag="mm")
                        _mm_chain(nc, ps,
                                  lambda c, k: wk_s[:, pr, c, :] if k == 1 else wk_s[:, pr, c:c + k, :],
                                  lambda c, k: XTw[:, c, :] if k == 1 else XTw[:, c:c + k, :],
                                  NCH, FP8_PROJ)
                        nc.vector.tensor_copy(KT[:, pr, tg * 512:(tg + 1) * 512], ps)

                        ps = mmp.tile([P, 512], F32, tag="mm")
                        _mm_chain(nc, ps,
                                  lambda c, k: wv_s[:, pr, c, :] if k == 1 else wv_s[:, pr, c:c + k, :],
                                  lambda c, k: XTw[:, c, :] if k == 1 else XTw[:, c:c + k, :],
                                  NCH, FP8_PROJ)
                        vt = scr.tile([P, 512], BF16, tag="vt")
                        nc.vector.tensor_copy(vt, ps)
                        if pending_vt is not None:
                            emit_vtrans(*pending_vt)
                        pending_vt = (vt, pr)
                    emit_vtrans(*pending_vt)

                    if tg % 2 == 1:
                        g = tg // 2
                        for pr in range(8):
                            ps = mmp.tile([P, 512], F32, tag="mm")
                            _mm_chain(nc, ps,
                                      lambda c, k: wq_s[:, pr, c, :] if k == 1 else wq_s[:, pr, c:c + k, :],
                                      lambda c, k: XQw[:, c, :] if k == 1 else XQw[:, c:c + k, :],
                                      NCH, FP8_PROJ)
                            nc.vector.tensor_copy(QT[:, pr, g * 512:(g + 1) * 512], ps)

            # ---- Phase 2: attention, head by head ----
            mid_cm = tc.tile_pool(name="mid", bufs=1)
            mid = mid_cm.__enter__()
            xv = mid.tile([P, NQ, D], F32)             # residual stream, my tokens
            for kb in range(NQ):
                nc.sync.dma_start(out=xv[:, kb, :], in_=xq[kb * P:(kb + 1) * P, :])
            with tc.tile_pool(name="ptp", bufs=3) as ptp, \
                 tc.tile_pool(name="stp", bufs=2, space="PSUM") as stp, \
                 tc.tile_pool(name="otp", bufs=2, space="PSUM") as otp, \
                 tc.tile_pool(name="trp", bufs=2, space="PSUM") as trp, \
                 tc.tile_pool(name="scr2", bufs=2) as scr2, \
                 tc.tile_pool(name="scr3", bufs=4) as scr3:
                for h in range(H):
                    pr, hh = h // 2, h % 2
                    hs = slice(hh * 64, (hh + 1) * 64)
                    OT = otp.tile([65, TQ], F32, tag="ot")
                    ot_sb = scr2.tile([65, TQ], F32, tag="otsb")

                    def drain(lo):
                        nc.vector.tensor_copy(ot_sb[:, lo:lo + 512], OT[:, lo:lo + 512])
                        for kb in range(lo // P, lo // P + 4):
                            o_ps = trp.tile([P, 65], F32, tag="otr")
                            nc.tensor.transpose(o_ps, ot_sb[:, kb * P:(kb + 1) * P],
                                                id_f32[0:65, 0:65])
                            rd = scr3.tile([P, 1], F32, tag="rd")
                            nc.vector.reciprocal(rd, o_ps[:, 64:65])
                            nc.vector.tensor_scalar_mul(ACC[:, kb, h * HD:(h + 1) * HD],
                                                        o_ps[:, 0:64], rd)

                    def emit_av(pt_j, j):
                        # AV accumulate, split at the PSUM bank boundary (col 512)
                        base = 64 * j
                        segs = ([(base, 512)] if base < 512 else []) + [(max(base, 512), TQ)]
                        for (s0, s1) in segs:
                            nc.tensor.matmul(OT[:, s0:s1], lhsT=VA[:, pr * 2 + hh, j, :],
                                             rhs=pt_j[:, s0 - base:s1 - base],
                                             start=(j == 0),
                                             stop=(j == 7 and s1 == 512) or (j == 15),
                                             skip_group_check=True)
                        if j == 7:
                            drain(0)

                    pending = None  # scores of j+1 are emitted before AV of j
                    for j in range(NB):
                        slen = TQ - 64 * j
                        base = 64 * j
                        pt_j = ptp.tile([P, TQ], BF16, tag="pt")
                        pos = 0
                        while pos < slen:
                            w = min(512, slen - pos)
                            st = stp.tile([P, 512], F32, tag="st")
                            nc.tensor.matmul(st[:, 0:w], lhsT=KT[hs, pr, j * P:(j + 1) * P],
                                             rhs=QT[hs, pr, base + pos:base + pos + w],
                                             start=True, stop=True)
                            if pos == 0:
                                nc.vector.tensor_add(st[:, 0:64], st[:, 0:64], mask_sb)
                            nc.scalar.activation(out=pt_j[:, pos:pos + w],
                                                 in_=st[:, 0:w], func=Exp, scale=sc_exp)
                            pos += w
                        if pending is not None:
                            emit_av(*pending)
                        pending = (pt_j, j)
                    emit_av(*pending)
                    drain(512)

            # ---- Phase 3: LN2 + MLP + residual, per 512-token group ----
            with tc.tile_pool(name="x2tp", bufs=1) as x2tp, \
                 tc.tile_pool(name="h1p", bufs=1) as h1p, \
                 tc.tile_pool(name="w1s", bufs=3) as w1s, \
                 tc.tile_pool(name="w2s", bufs=2) as w2s, \
                 tc.tile_pool(name="lnp2", bufs=2) as lnp2, \
                 tc.tile_pool(name="scr4", bufs=2) as scr4, \
                 tc.tile_pool(name="mmd", bufs=3, space="PSUM") as mmd, \
                 tc.tile_pool(name="trd", bufs=2, space="PSUM") as trd:
                for g in range(2):
                    X2T = x2tp.tile([P, NCH, 512], mdt, tag="x2t")
                    for s in range(4):
                        kb = g * 4 + s
                        nc.vector.tensor_add(xv[:, kb, :], xv[:, kb, :], ACC[:, kb, :])
                        xn = layernorm_rows(lnp2, xv[:, kb, :], g2_t, be2_t)
                        transpose_to(trd, xn, X2T, s * P, mdt)
                    h1 = h1p.tile([P, 32, 512], mdt, tag="h1")
                    for f in range(32):
                        w1f = w1s.tile([P, NCH, P], mdt, tag="w1f")
                        nc.sync.dma_start(out=w1f, in_=w1t[f])
                        ps = mmd.tile([P, 512], F32, tag="mm")
                        _mm_chain(nc, ps,
                                  lambda c, k: w1f[:, c, :] if k == 1 else w1f[:, c:c + k, :],
                                  lambda c, k: X2T[:, c, :] if k == 1 else X2T[:, c:c + k, :],
                                  NCH, FP8_MLP)
                        nc.scalar.activation(out=h1[:, f, :], in_=ps, func=Relu,
                                             bias=b1_sb[:, f:f + 1], scale=1.0)
                    # W2 stage; output transposes lag one dd so the PE never
                    # waits on the DVE bias-add of fsb.
                    def emit_out(fsb, dd):
                        for s in range(4):
                            kb = g * 4 + s
                            tp = trd.tile([P, 4, P], F32, tag="lntr")
                            nc.tensor.matmul(tp[:, 0, :], lhsT=fsb[:, s * P:(s + 1) * P],
                                             rhs=id_f32, is_transpose=True,
                                             start=True, stop=True)
                            nc.vector.tensor_add(xv[:, kb, dd * P:(dd + 1) * P],
                                                 xv[:, kb, dd * P:(dd + 1) * P],
                                                 tp[:, 0, :])

                    pending_o = None
                    for dd in range(8):
                        w2d = w2s.tile([P, 32, P], mdt, tag="w2d")
                        nc.sync.dma_start(out=w2d, in_=w2t[dd])
                        ps = mmd.tile([P, 512], F32, tag="mm")
                        _mm_chain(nc, ps,
                                  lambda c, k: w2d[:, c, :] if k == 1 else w2d[:, c:c + k, :],
                                  lambda c, k: h1[:, c, :] if k == 1 else h1[:, c:c + k, :],
                                  32, FP8_MLP)
                        fsb = scr4.tile([P, 512], F32, tag="fsb")
                        if FP8_MLP:
                            nc.vector.tensor_scalar(out=fsb, in0=ps, scalar1=inv_sb,
                                                    scalar2=b2_sb[:, dd:dd + 1],
                                                    op0=mybir.AluOpType.mult,
                                                    op1=mybir.AluOpType.add)
                        else:
                            nc.vector.tensor_scalar_add(fsb, ps, b2_sb[:, dd:dd + 1])
                        if pending_o is not None:
                            emit_out(*pending_o)
                        pending_o = (fsb, dd)
                    emit_out(*pending_o)
                    for s in range(4):
                        kb = g * 4 + s
                        nc.sync.dma_start(out=out_d[kb * P:(kb + 1) * P, :],
                                          in_=xv[:, kb, :])
            mid_cm.__exit__(None, None, None)

    _split_drain_waits(nc)
    return nc


def _split_drain_waits(nc):
    """This walrus build gives every instruction a single hardware wait slot
    (one EVENTS struct per 64B instruction). Tile emits multi-wait
    instructions; move the excess waits onto single-wait NoOps inserted just
    before, on the same engine — identical semantics in program order."""
    for fn in nc.m.functions:
        for blk in fn.blocks:
            insts = blk.instructions
            i = 0
            while i < len(insts):
                inst = insts[i]
                si = inst.sync_info
                if si is not None and len(si.on_wait) > 1:
                    waits = list(si.on_wait)
                    inst.sync_info = mybir.SyncInfo(on_wait=[waits[-1]],
                                                    on_update=list(si.on_update))
                    for w in waits[:-1]:
                        nop = mybir.InstNoOp(name=nc.get_next_instruction_name(),
                                             ins=[], outs=[])
                        nop.engine = inst.engine
                        nop.sync_info = mybir.SyncInfo(on_wait=[w], on_update=[])
                        nc.register_instruction(nop, overwrite=True)
                        insts.insert(i, nop)
                        i += 1
                i += 1


def _prep_inputs(inputs, Wq, Wk, Wv, W1, b1, W2, b2, g1, be1, g2, be2,
                 apply_g1, apply_g2):
    bf = ml_dtypes.bfloat16
    f8 = ml_dtypes.float8_e4m3
    f32 = np.float32
    xdt = f8 if FP8_PROJ else bf
    mdt = f8 if FP8_MLP else bf
    pws = WS if FP8_PROJ else 1.0
    mws = MS if FP8_MLP else 1.0
    inputs = np.ascontiguousarray(np.asarray(inputs, f32))
    wq_f = np.asarray(Wq, f32).transpose(1, 0, 2).reshape(D, D) * pws
    wk_f = np.asarray(Wk, f32).transpose(1, 0, 2).reshape(D, D) * pws
    wv_f = np.asarray(Wv, f32).transpose(1, 0, 2).reshape(D, D) * pws

    def pair_tiles(w, dt):  # [D, D] -> [8, 128, 8, 128] (pair, p, chunk, col)
        return np.ascontiguousarray(
            w.reshape(NCH, P, 8, P).transpose(2, 1, 0, 3).astype(dt))

    wq_t = pair_tiles(wq_f, xdt)
    wk_t = pair_tiles(wk_f, xdt)
    wv_t = pair_tiles(wv_f, xdt)
    w1_t = np.ascontiguousarray(
        (np.asarray(W1, f32) * mws).reshape(NCH, P, 32, P).transpose(2, 1, 0, 3).astype(mdt))
    w2_t = np.ascontiguousarray(
        (np.asarray(W2, f32) * mws).reshape(32, P, 8, P).transpose(2, 1, 0, 3).astype(mdt))
    b1_t = np.ascontiguousarray((np.asarray(b1, f32) * mws).reshape(32, P).T)
    b2_t = np.ascontiguousarray(np.asarray(b2, f32).reshape(8, P).T)

    in_maps = []
    for c in range(8):
        b, o = divmod(c, 2)
        xb_c = inputs[b]
        xq_c = np.ascontiguousarray(xb_c[o::2, :])
        cc, kk = np.meshgrid(np.arange(P), np.arange(64), indexing="ij")
        mask = np.where(cc <= 2 * kk + o, 0.0, NEG * pws * pws).astype(f32)
        m = {"xb": xb_c, "xq": xq_c, "wq": wq_t, "wk": wk_t, "wv": wv_t,
             "w1t": w1_t, "w2t": w2_t, "b1t": b1_t, "b2t": b2_t, "maskt": mask}
        if apply_g1:
            m["g1v"] = np.asarray(g1, f32)
            m["be1v"] = np.asarray(be1, f32)
        if apply_g2:
            m["g2v"] = np.asarray(g2, f32)
            m["be2v"] = np.asarray(be2, f32)
        in_maps.append(m)
    return in_maps


def _run(inputs, Wq, Wk, Wv, W1, b1, W2, b2, g1, be1, g2, be2, **spmd_kwargs):
    apply_g1 = not (np.all(np.asarray(g1) == 1.0) and np.all(np.asarray(be1) == 0.0))
    apply_g2 = not (np.all(np.asarray(g2) == 1.0) and np.all(np.asarray(be2) == 0.0))
    nc = build_program(apply_g1, apply_g2)
    in_maps = _prep_inputs(inputs, Wq, Wk, Wv, W1, b1, W2, b2, g1, be1, g2, be2,
                           apply_g1, apply_g2)
    res = run_bass_kernel_spmd(nc, in_maps, list(range(8)), **spmd_kwargs)
    out = np.empty((B, T, D), np.float32)
    for c in range(8):
        b, o = divmod(c, 2)
        out[b, o::2, :] = res.results[c]["out"]
    return out, res


def kernel(inputs, Wq, Wk, Wv, W1, b1, W2, b2, g1, be1, g2, be2):
    out, _ = _run(inputs, Wq, Wk, Wv, W1, b1, W2, b2, g1, be1, g2, be2)
    return out


# revision 23
# speedup vs baseline: 1.1624x; 1.1624x over previous
"""GPT block (LN -> causal MHA -> residual -> LN -> MLP -> residual) on 8 trn2 cores.

Sharding: core c = (batch b = c//2, parity o = c%2). Each core owns the
interleaved tokens o::2 of its batch. K/V are computed redundantly by the two
cores of a batch; attention rows and the MLP are token-parallel. No cross-core
communication: outputs are scattered back on the host.

v2 structure (keeps the PE dense so the HAM clock gate stays at 8/8):
 - Phase 1 interleaves LN1 (DVE) with the QKV projections (PE) in 512-token
   windows; all pairs' K^T / Q^T / V-augmented tiles stay resident in SBUF.
 - Phase 2 runs attention head-by-head: per key-block j, scores -> exp -> AV
   accumulate into a persistent [65, 1024] PSUM tile (extra ones-row gives the
   softmax denominator). Scores of block j+1 overlap the exp/AV of block j.
 - Phase 3 is the MLP, token-parallel, unchanged in shape.

Matmuls run in bf16 (fp32 PSUM), optionally fp8 e4m3 with DoubleRow
(2x contraction/cycle) for the projections and the MLP; layernorm/softmax
stats stay fp32.
"""

import sys

if "/opt/trn_rl_repo" not in sys.path:
    sys.path.insert(0, "/opt/trn_rl_repo")

import numpy as np
import ml_dtypes

import concourse.bass as bass
import concourse.tile as tile
from concourse import mybir
from concourse.bass_utils import run_bass_kernel_spmd
from concourse.masks import make_identity

B, T, D, H, HD = 4, 2048, 1024, 16, 64
FF = 4 * D
P = 128
NB = T // P        # 16 key blocks
TQ = T // 2        # 1024 query tokens per core
NQ = TQ // P       # 8 query blocks per core
NCH = D // P       # 8 contraction chunks over D
EPS = 1e-5
F32 = mybir.dt.float32
BF16 = mybir.dt.bfloat16
FP8 = mybir.dt.float8e4
DR = mybir.MatmulPerfMode.DoubleRow

FP8_PROJ = False   # QKV projections in fp8 DoubleRow
FP8_MLP = False    # MLP matmuls in fp8 DoubleRow
WS = 32.0          # fp8 weight scale (proj)
MS = 32.0          # fp8 weight scale (mlp)
NEG = -240.0       # exp(0.125*(s-240)) ~ 1e-12: exact-enough zero


def _mm_chain(nc, ps, w_ap, x_ap, nch, fp8):
    """Accumulating matmul chain over `nch` 128-chunks of the contraction.
    w_ap(c, k) / x_ap(c, k) return APs for chunk slice [c, c+k)."""
    if fp8:
        for c in range(0, nch, 2):
            nc.tensor.matmul(ps, lhsT=w_ap(c, 2), rhs=x_ap(c, 2),
                             start=(c == 0), stop=(c == nch - 2), perf_mode=DR)
    else:
        for c in range(nch):
            nc.tensor.matmul(ps, lhsT=w_ap(c, 1), rhs=x_ap(c, 1),
                             start=(c == 0), stop=(c == nch - 1))


def build_program(apply_g1=False, apply_g2=False):
    nc = bass.Bass()
    xdt = FP8 if FP8_PROJ else BF16
    mdt = FP8 if FP8_MLP else BF16
    sc_exp = 0.125 / (WS * WS if FP8_PROJ else 1.0)
    aug = WS if FP8_PROJ else 1.0

    xb = nc.declare_dram_parameter("xb", [T, D], F32, isOutput=False)
    xq = nc.declare_dram_parameter("xq", [TQ, D], F32, isOutput=False)
    wq = nc.declare_dram_parameter("wq", [8, P, NCH, P], xdt, isOutput=False)
    wk = nc.declare_dram_parameter("wk", [8, P, NCH, P], xdt, isOutput=False)
    wv = nc.declare_dram_parameter("wv", [8, P, NCH, P], xdt, isOutput=False)
    w1t = nc.declare_dram_parameter("w1t", [32, P, NCH, P], mdt, isOutput=False)
    w2t = nc.declare_dram_parameter("w2t", [8, P, 32, P], mdt, isOutput=False)
    b1t = nc.declare_dram_parameter("b1t", [P, 32], F32, isOutput=False)
    b2t = nc.declare_dram_parameter("b2t", [P, 8], F32, isOutput=False)
    maskt = nc.declare_dram_parameter("maskt", [P, 64], F32, isOutput=False)
    gb = {}
    if apply_g1:
        gb["g1"] = nc.declare_dram_parameter("g1v", [D], F32, isOutput=False)
        gb["be1"] = nc.declare_dram_parameter("be1v", [D], F32, isOutput=False)
    if apply_g2:
        gb["g2"] = nc.declare_dram_parameter("g2v", [D], F32, isOutput=False)
        gb["be2"] = nc.declare_dram_parameter("be2v", [D], F32, isOutput=False)
    out_d = nc.declare_dram_parameter("out", [TQ, D], F32, isOutput=True)

    Exp = mybir.ActivationFunctionType.Exp
    Relu = mybir.ActivationFunctionType.Relu

    with tile.TileContext(nc) as tc:
        with tc.tile_pool(name="consts", bufs=1) as consts, \
             tc.tile_pool(name="big", bufs=1) as big:
            id_f32 = consts.tile([P, P], F32)
            make_identity(nc, id_f32)
            id_bf = consts.tile([P, P], BF16)
            make_identity(nc, id_bf)
            eps_sb = consts.tile([P, 1], F32)
            nc.vector.memset(eps_sb, EPS)
            mask_sb = consts.tile([P, 64], F32)
            nc.sync.dma_start(out=mask_sb, in_=maskt[:, :])
            b1_sb = consts.tile([P, 32], F32)
            nc.sync.dma_start(out=b1_sb, in_=b1t[:, :])
            b2_sb = consts.tile([P, 8], F32)
            nc.sync.dma_start(out=b2_sb, in_=b2t[:, :])
            inv_sb = consts.tile([P, 1], F32)
            nc.vector.memset(inv_sb, 1.0 / (MS * MS))

            def bcast(name):
                t = consts.tile([P, D], F32, tag=f"bc_{name}")
                src = gb[name]
                ap = bass.AP(tensor=src.tensor if hasattr(src, "tensor") else src[:].tensor,
                             offset=src[:].offset, ap=[[0, P]] + list(src[:].ap))
                nc.sync.dma_start(out=t, in_=ap)
                return t

            g1_t = bcast("g1") if apply_g1 else None
            be1_t = bcast("be1") if apply_g1 else None
            g2_t = bcast("g2") if apply_g2 else None
            be2_t = bcast("be2") if apply_g2 else None

            # ---- resident tensors ----
            XT = big.tile([P, NCH, T], xdt)            # LN1(xb)^T
            XQT = big.tile([P, NCH, TQ], xdt)          # LN1(xq)^T
            ACC = big.tile([P, NQ, D], BF16)           # attention out accumulator

            def layernorm_rows(lnp, src_ap, gt, bt):
                """LN over rows of src_ap [128, D] f32 -> xn [128, D] f32."""
                stats = lnp.tile([P, 2, 6], F32, tag="stats")
                for s in range(2):
                    nc.vector.bn_stats(out=stats[:, s, :], in_=src_ap[:, s * 512:(s + 1) * 512])
                mv = lnp.tile([P, 2], F32, tag="mv")
                nc.vector.bn_aggr(out=mv, in_=stats)
                rstd = lnp.tile([P, 1], F32, tag="rstd")
                nc.scalar.activation(out=rstd, in_=mv[:, 1:2],
                                     func=mybir.ActivationFunctionType.Sqrt,
                                     bias=eps_sb, scale=1.0)
                nc.vector.reciprocal(out=rstd, in_=rstd)
                xn = lnp.tile([P, D], F32, tag="xn")
                nc.vector.tensor_scalar(out=xn, in0=src_ap, scalar1=mv[:, 0:1],
                                        scalar2=rstd, op0=mybir.AluOpType.subtract,
                                        op1=mybir.AluOpType.mult)
                if gt is not None:
                    nc.vector.tensor_mul(xn, xn, gt)
                if bt is not None:
                    nc.vector.tensor_add(xn, xn, bt)
                return xn

            def transpose_to(pst, xn, dst3, col, dtype, tag="lntr"):
                """xn [128, D] -> transposed bf16/fp8 into dst3[:, c, col:col+128]."""
                for half in range(2):
                    ps = pst.tile([P, 4, P], F32, tag=tag)
                    for cc in range(4):
                        c = half * 4 + cc
                        nc.tensor.matmul(ps[:, cc, :], lhsT=xn[:, c * P:(c + 1) * P],
                                         rhs=id_f32, is_transpose=True,
                                         start=(cc == 0), stop=(cc == 3),
                                         skip_group_check=True)
                    nc.vector.tensor_copy(dst3[:, half * 4:(half + 1) * 4, col:col + P], ps)

            # ---- Phases 1+2: LN1, then per pair: attention braided with the
            # next pair's QKV projections (dense chains keep the PE clock
            # gate warm through the attention's gappier stream) ----
            mid_cm = tc.tile_pool(name="mid", bufs=1)
            mid = mid_cm.__enter__()
            xv = mid.tile([P, NQ, D], F32)             # residual stream, my tokens
            with tc.tile_pool(name="wpp", bufs=2) as wpp, \
                 tc.tile_pool(name="kqv", bufs=2) as kqv, \
                 tc.tile_pool(name="lnsrc", bufs=3) as lnsrc, \
                 tc.tile_pool(name="lnp", bufs=3) as lnp, \
                 tc.tile_pool(name="scr", bufs=3) as scr, \
                 tc.tile_pool(name="ptp", bufs=3) as ptp, \
                 tc.tile_pool(name="scr2", bufs=2) as scr2, \
                 tc.tile_pool(name="scr3", bufs=4) as scr3, \
                 tc.tile_pool(name="mmp", bufs=4, space="PSUM") as mmp, \
                 tc.tile_pool(name="otp", bufs=1, space="PSUM") as otp, \
                 tc.tile_pool(name="trp", bufs=2, space="PSUM") as trp:

                def start_pair(p):
                    """DMA pair p's weights, allocate its K^T/Q^T/V tiles, and
                    return emit-closures for its projection chains."""
                    wqp = wpp.tile([P, NCH, P], xdt, tag="wq")
                    nc.sync.dma_start(out=wqp, in_=wq[p])
                    wkp = wpp.tile([P, NCH, P], xdt, tag="wk")
                    nc.sync.dma_start(out=wkp, in_=wk[p])
                    wvp = wpp.tile([P, NCH, P], xdt, tag="wv")
                    nc.sync.dma_start(out=wvp, in_=wv[p])
                    kt = kqv.tile([P, T], BF16, tag="kt")
                    qt = kqv.tile([P, TQ], BF16, tag="qt")
                    va = kqv.tile([P, 2, NB, 65], BF16, tag="va")
                    nc.vector.memset(va[:, :, :, 64:65], aug)
                    vts = {}

                    def kchain(tg):
                        ps = mmp.tile([P, 512], F32, tag="mm")
                        _mm_chain(nc, ps,
                                  lambda c, k: wkp[:, c, :] if k == 1 else wkp[:, c:c + k, :],
                                  lambda c, k: XT[:, c, tg * 512:(tg + 1) * 512] if k == 1
                                  else XT[:, c:c + k, tg * 512:(tg + 1) * 512],
                                  NCH, FP8_PROJ)
                        nc.vector.tensor_copy(kt[:, tg * 512:(tg + 1) * 512], ps)

                    def vchain(tg):
                        ps = mmp.tile([P, 512], F32, tag="mm")
                        _mm_chain(nc, ps,
                                  lambda c, k: wvp[:, c, :] if k == 1 else wvp[:, c:c + k, :],
                                  lambda c, k: XT[:, c, tg * 512:(tg + 1) * 512] if k == 1
                                  else XT[:, c:c + k, tg * 512:(tg + 1) * 512],
                                  NCH, FP8_PROJ)
                        vt = scr.tile([P, 512], BF16, tag="vt")
                        nc.vector.tensor_copy(vt, ps)
                        vts[tg] = vt

                    def vtrans(tg):
                        vt = vts.pop(tg)
                        for hh in range(2):
                            tps = trp.tile([P, 4, 66], BF16, tag="tr")
                            for s in range(4):
                                nc.tensor.matmul(
                                    tps[:, s, 0:64],
                                    lhsT=vt[hh * 64:(hh + 1) * 64, s * P:(s + 1) * P],
                                    rhs=id_bf[hh * 64:hh * 64 + 64, hh * 64:hh * 64 + 64],
                                    is_transpose=True, start=(s == 0), stop=(s == 3),
                                    skip_group_check=True)
                            nc.vector.tensor_copy(va[:, hh, tg * 4:(tg + 1) * 4, 0:64],
                                                  tps[:, :, 0:64])

                    def qchain(g):
                        ps = mmp.tile([P, 512], F32, tag="mm")
                        _mm_chain(nc, ps,
                                  lambda c, k: wqp[:, c, :] if k == 1 else wqp[:, c:c + k, :],
                                  lambda c, k: XQT[:, c, g * 512:(g + 1) * 512] if k == 1
                                  else XQT[:, c:c + k, g * 512:(g + 1) * 512],
                                  NCH, FP8_PROJ)
                        nc.vector.tensor_copy(qt[:, g * 512:(g + 1) * 512], ps)

                    def em(f, *a):
                        return lambda: f(*a)
                    ems = [em(kchain, 0), em(vchain, 0), em(kchain, 1), em(vtrans, 0),
                           em(vchain, 1), em(kchain, 2), em(vtrans, 1), em(vchain, 2),
                           em(kchain, 3), em(vtrans, 2), em(vchain, 3), em(qchain, 0),
                           em(vtrans, 3), em(qchain, 1)]
                    return (kt, qt, va), ems

                def attention_pair(p, tiles, braid):
                    kt, qt, va = tiles
                    bi = [0]

                    def pop():
                        if bi[0] < len(braid):
                            braid[bi[0]]()
                            bi[0] += 1

                    for hh in range(2):
                        h = 2 * p + hh
                        hs = slice(hh * 64, (hh + 1) * 64)
                        OT = otp.tile([65, TQ], F32, tag="ot")
                        ot_sb = scr2.tile([65, TQ], F32, tag="otsb")

                        def drain(lo):
                            nc.vector.tensor_copy(ot_sb[:, lo:lo + 512], OT[:, lo:lo + 512])
                            for kb in range(lo // P, lo // P + 4):
                                o_ps = trp.tile([P, 65], F32, tag="tr")
                                nc.tensor.transpose(o_ps, ot_sb[:, kb * P:(kb + 1) * P],
                                                    id_f32[0:65, 0:65])
                                rd = scr3.tile([P, 1], F32, tag="rd")
                                nc.vector.reciprocal(rd, o_ps[:, 64:65])
                                nc.vector.tensor_scalar_mul(
                                    ACC[:, kb, h * HD:(h + 1) * HD], o_ps[:, 0:64], rd)

                        def emit_av(pt_j, j):
                            # split at the PSUM bank boundary (col 512)
                            base = 64 * j
                            segs = (([(base, 512)] if base < 512 else [])
                                    + [(max(base, 512), TQ)])
                            for (s0, s1) in segs:
                                nc.tensor.matmul(OT[:, s0:s1], lhsT=va[:, hh, j, :],
                                                 rhs=pt_j[:, s0 - base:s1 - base],
                                                 start=(j == 0),
                                                 stop=(j == 7 and s1 == 512) or (j == 15),
                                                 skip_group_check=True)
                            if j == 7:
                                drain(0)

                        pending = None  # scores of j+1 are emitted before AV of j
                        for j in range(NB):
                            slen = TQ - 64 * j
                            base = 64 * j
                            pt_j = ptp.tile([P, TQ], BF16, tag="pt")
                            pos = 0
                            while pos < slen:
                                w = min(512, slen - pos)
                                st = mmp.tile([P, 512], F32, tag="mm")
                                nc.tensor.matmul(st[:, 0:w],
                                                 lhsT=kt[hs, j * P:(j + 1) * P],
                                                 rhs=qt[hs, base + pos:base + pos + w],
                                                 start=True, stop=True)
                                if pos == 0:
                                    nc.vector.tensor_add(st[:, 0:64], st[:, 0:64], mask_sb)
                                nc.scalar.activation(out=pt_j[:, pos:pos + w],
                                                     in_=st[:, 0:w], func=Exp, scale=sc_exp)
                                pos += w
                            if pending is not None:
                                emit_av(*pending)
                            pending = (pt_j, j)
                            if j % 2 == 0:
                                pop()
                        emit_av(*pending)
                        drain(512)
                        pop()
                    while bi[0] < len(braid):
                        braid[bi[0]]()
                        bi[0] += 1

                # LN1 for xb and xq, braided with pair 0's projections
                tiles0, ems0 = start_pair(0)
                e0 = 0
                for tg in range(4):
                    for bi2 in range(4):
                        blk = tg * 4 + bi2
                        x_t = lnsrc.tile([P, D], F32, tag="xsrc")
                        nc.sync.dma_start(out=x_t, in_=xb[blk * P:(blk + 1) * P, :])
                        xn = layernorm_rows(lnp, x_t, g1_t, be1_t)
                        transpose_to(mmp, xn, XT, tg * 512 + bi2 * P, xdt, tag="mm")
                    for qi in range(2):
                        kb = tg * 2 + qi
                        xq_t = lnsrc.tile([P, D], F32, tag="xsrc")
                        nc.sync.dma_start(out=xq_t, in_=xq[kb * P:(kb + 1) * P, :])
                        xn = layernorm_rows(lnp, xq_t, g1_t, be1_t)
                        transpose_to(mmp, xn, XQT, kb * P, xdt, tag="mm")
                    # pair-0 chains for windows that are now complete
                    while e0 < [2, 5, 8, 14][tg]:
                        ems0[e0]()
                        e0 += 1
                for kb in range(NQ):
                    nc.sync.dma_start(out=xv[:, kb, :], in_=xq[kb * P:(kb + 1) * P, :])

                tiles, ems = tiles0, ems0
                for p in range(8):
                    nxt = start_pair(p + 1) if p < 7 else (None, [])
                    attention_pair(p, tiles, nxt[1])
                    tiles = nxt[0]
            # ---- Phase 3: LN2 + MLP + residual, per 512-token group ----
            with tc.tile_pool(name="x2tp", bufs=1) as x2tp, \
                 tc.tile_pool(name="h1p", bufs=1) as h1p, \
                 tc.tile_pool(name="w1s", bufs=3) as w1s, \
                 tc.tile_pool(name="w2s", bufs=2) as w2s, \
                 tc.tile_pool(name="lnp2", bufs=2) as lnp2, \
                 tc.tile_pool(name="scr4", bufs=2) as scr4, \
                 tc.tile_pool(name="mmd", bufs=3, space="PSUM") as mmd, \
                 tc.tile_pool(name="trd", bufs=2, space="PSUM") as trd:
                for g in range(2):
                    X2T = x2tp.tile([P, NCH, 512], mdt, tag="x2t")
                    for s in range(4):
                        kb = g * 4 + s
                        nc.vector.tensor_add(xv[:, kb, :], xv[:, kb, :], ACC[:, kb, :])
                        xn = layernorm_rows(lnp2, xv[:, kb, :], g2_t, be2_t)
                        transpose_to(trd, xn, X2T, s * P, mdt)
                    h1 = h1p.tile([P, 32, 512], mdt, tag="h1")
                    for f in range(32):
                        w1f = w1s.tile([P, NCH, P], mdt, tag="w1f")
                        nc.sync.dma_start(out=w1f, in_=w1t[f])
                        ps = mmd.tile([P, 512], F32, tag="mm")
                        _mm_chain(nc, ps,
                                  lambda c, k: w1f[:, c, :] if k == 1 else w1f[:, c:c + k, :],
                                  lambda c, k: X2T[:, c, :] if k == 1 else X2T[:, c:c + k, :],
                                  NCH, FP8_MLP)
                        nc.scalar.activation(out=h1[:, f, :], in_=ps, func=Relu,
                                             bias=b1_sb[:, f:f + 1], scale=1.0)
                    # W2 stage; output transposes lag one dd so the PE never
                    # waits on the DVE bias-add of fsb.
                    def emit_out(fsb, dd):
                        for s in range(4):
                            kb = g * 4 + s
                            tp = trd.tile([P, 4, P], F32, tag="lntr")
                            nc.tensor.matmul(tp[:, 0, :], lhsT=fsb[:, s * P:(s + 1) * P],
                                             rhs=id_f32, is_transpose=True,
                                             start=True, stop=True)
                            nc.vector.tensor_add(xv[:, kb, dd * P:(dd + 1) * P],
                                                 xv[:, kb, dd * P:(dd + 1) * P],
                                                 tp[:, 0, :])

                    pending_o = None
                    for dd in range(8):
                        w2d = w2s.tile([P, 32, P], mdt, tag="w2d")
                        nc.sync.dma_start(out=w2d, in_=w2t[dd])
                        ps = mmd.tile([P, 512], F32, tag="mm")
                        _mm_chain(nc, ps,
                                  lambda c, k: w2d[:, c, :] if k == 1 else w2d[:, c:c + k, :],
                                  lambda c, k: h1[:, c, :] if k == 1 else h1[:, c:c + k, :],
                                  32, FP8_MLP)
                        fsb = scr4.tile([P, 512], F32, tag="fsb")
                        if FP8_MLP:
                            nc.vector.tensor_scalar(out=fsb, in0=ps, scalar1=inv_sb,
                                                    scalar2=b2_sb[:, dd:dd + 1],
                                                    op0=mybir.AluOpType.mult,
                                                    op1=mybir.AluOpType.add)
                        else:
                            nc.vector.tensor_scalar_add(fsb, ps, b2_sb[:, dd:dd + 1])
                        if pending_o is not None:
                            emit_out(*pending_o)
                        pending_o = (fsb, dd)
                    emit_out(*pending_o)
                    for s in range(4):
                        kb = g * 4 + s
                        nc.sync.dma_start(out=out_d[kb * P:(kb + 1) * P, :],
                                          in_=xv[:, kb, :])
            mid_cm.__exit__(None, None, None)

    _split_drain_waits(nc)
    return nc


def _split_drain_waits(nc):
    """This walrus build gives every instruction a single hardware wait slot
    (one EVENTS struct per 64B instruction). Tile emits multi-wait
    instructions; move the excess waits onto single-wait NoOps inserted just
    before, on the same engine — identical semantics in program order."""
    for fn in nc.m.functions:
        for blk in fn.blocks:
            insts = blk.instructions
            i = 0
            while i < len(insts):
                inst = insts[i]
                si = inst.sync_info
                if si is not None and len(si.on_wait) > 1:
                    waits = list(si.on_wait)
                    inst.sync_info = mybir.SyncInfo(on_wait=[waits[-1]],
                                                    on_update=list(si.on_update))
                    for w in waits[:-1]:
                        nop = mybir.InstNoOp(name=nc.get_next_instruction_name(),
                                             ins=[], outs=[])
                        nop.engine = inst.engine
                        nop.sync_info = mybir.SyncInfo(on_wait=[w], on_update=[])
                        nc.register_instruction(nop, overwrite=True)
                        insts.insert(i, nop)
                        i += 1
                i += 1


def _prep_inputs(inputs, Wq, Wk, Wv, W1, b1, W2, b2, g1, be1, g2, be2,
                 apply_g1, apply_g2):
    bf = ml_dtypes.bfloat16
    f8 = ml_dtypes.float8_e4m3
    f32 = np.float32
    xdt = f8 if FP8_PROJ else bf
    mdt = f8 if FP8_MLP else bf
    pws = WS if FP8_PROJ else 1.0
    mws = MS if FP8_MLP else 1.0
    inputs = np.ascontiguousarray(np.asarray(inputs, f32))
    wq_f = np.asarray(Wq, f32).transpose(1, 0, 2).reshape(D, D) * pws
    wk_f = np.asarray(Wk, f32).transpose(1, 0, 2).reshape(D, D) * pws
    wv_f = np.asarray(Wv, f32).transpose(1, 0, 2).reshape(D, D) * pws

    def pair_tiles(w, dt):  # [D, D] -> [8, 128, 8, 128] (pair, p, chunk, col)
        return np.ascontiguousarray(
            w.reshape(NCH, P, 8, P).transpose(2, 1, 0, 3).astype(dt))

    wq_t = pair_tiles(wq_f, xdt)
    wk_t = pair_tiles(wk_f, xdt)
    wv_t = pair_tiles(wv_f, xdt)
    w1_t = np.ascontiguousarray(
        (np.asarray(W1, f32) * mws).reshape(NCH, P, 32, P).transpose(2, 1, 0, 3).astype(mdt))
    w2_t = np.ascontiguousarray(
        (np.asarray(W2, f32) * mws).reshape(32, P, 8, P).transpose(2, 1, 0, 3).astype(mdt))
    b1_t = np.ascontiguousarray((np.asarray(b1, f32) * mws).reshape(32, P).T)
    b2_t = np.ascontiguousarray(np.asarray(b2, f32).reshape(8, P).T)

    in_maps = []
    for c in range(8):
        b, o = divmod(c, 2)
        xb_c = inputs[b]
        xq_c = np.ascontiguousarray(xb_c[o::2, :])
        cc, kk = np.meshgrid(np.arange(P), np.arange(64), indexing="ij")
        mask = np.where(cc <= 2 * kk + o, 0.0, NEG * pws * pws).astype(f32)
        m = {"xb": xb_c, "xq": xq_c, "wq": wq_t, "wk": wk_t, "wv": wv_t,
             "w1t": w1_t, "w2t": w2_t, "b1t": b1_t, "b2t": b2_t, "maskt": mask}
        if apply_g1:
            m["g1v"] = np.asarray(g1, f32)
            m["be1v"] = np.asarray(be1, f32)
        if apply_g2:
            m["g2v"] = np.asarray(g2, f32)
            m["be2v"] = np.asarray(be2, f32)
        in_maps.append(m)
    return in_maps


def _run(inputs, Wq, Wk, Wv, W1, b1, W2, b2, g1, be1, g2, be2, **spmd_kwargs):
    apply_g1 = not (np.all(np.asarray(g1) == 1.0) and np.all(np.asarray(be1) == 0.0))
    apply_g2 = not (np.all(np.asarray(g2) == 1.0) and np.all(np.asarray(be2) == 0.0))
    nc = build_program(apply_g1, apply_g2)
    in_maps = _prep_inputs(inputs, Wq, Wk, Wv, W1, b1, W2, b2, g1, be1, g2, be2,
                           apply_g1, apply_g2)
    res = run_bass_kernel_spmd(nc, in_maps, list(range(8)), **spmd_kwargs)
    out = np.empty((B, T, D), np.float32)
    for c in range(8):
        b, o = divmod(c, 2)
        out[b, o::2, :] = res.results[c]["out"]
    return out, res


def kernel(inputs, Wq, Wk, Wv, W1, b1, W2, b2, g1, be1, g2, be2):
    out, _ = _run(inputs, Wq, Wk, Wv, W1, b1, W2, b2, g1, be1, g2, be2)
    return out


# revision 31
# speedup vs baseline: 1.3100x; 1.1269x over previous
"""GPT block (LN -> causal MHA -> residual -> LN -> MLP -> residual) on 8 trn2 cores.

Sharding: core c = (batch b = c//2, parity o = c%2). Each core owns the
interleaved tokens o::2 of its batch. K/V are computed redundantly by the two
cores of a batch; attention rows and the MLP are token-parallel. No cross-core
communication: outputs are scattered back on the host.

v2 structure (keeps the PE dense so the HAM clock gate stays at 8/8):
 - Phase 1 interleaves LN1 (DVE) with the QKV projections (PE) in 512-token
   windows; all pairs' K^T / Q^T / V-augmented tiles stay resident in SBUF.
 - Phase 2 runs attention head-by-head: per key-block j, scores -> exp -> AV
   accumulate into a persistent [65, 1024] PSUM tile (extra ones-row gives the
   softmax denominator). Scores of block j+1 overlap the exp/AV of block j.
 - Phase 3 is the MLP, token-parallel, unchanged in shape.

Matmuls run in bf16 (fp32 PSUM), optionally fp8 e4m3 with DoubleRow
(2x contraction/cycle) for the projections and the MLP; layernorm/softmax
stats stay fp32.
"""

import sys

if "/opt/trn_rl_repo" not in sys.path:
    sys.path.insert(0, "/opt/trn_rl_repo")

import numpy as np
import ml_dtypes

import concourse.bass as bass
import concourse.tile as tile
from concourse import mybir
from concourse.bass_utils import run_bass_kernel_spmd
from concourse.masks import make_identity

B, T, D, H, HD = 4, 2048, 1024, 16, 64
FF = 4 * D
P = 128
NB = T // P        # 16 key blocks
TQ = T // 2        # 1024 query tokens per core
NQ = TQ // P       # 8 query blocks per core
NCH = D // P       # 8 contraction chunks over D
EPS = 1e-5
F32 = mybir.dt.float32
BF16 = mybir.dt.bfloat16
FP8 = mybir.dt.float8e4
DR = mybir.MatmulPerfMode.DoubleRow

FP8_PROJ = True    # QKV projections in fp8 DoubleRow
FP8_MLP = False    # MLP matmuls in fp8 DoubleRow (numerically out of budget)
WS = 32.0          # fp8 weight scale (proj)
MS = 32.0          # fp8 weight scale (mlp)
NEG = -240.0       # exp(0.125*(s-240)) ~ 1e-12: exact-enough zero


def _mm_chain(nc, ps, w_ap, x_ap, nch, fp8):
    """Accumulating matmul chain over `nch` 128-chunks of the contraction.
    w_ap(c, k) / x_ap(c, k) return APs for chunk slice [c, c+k)."""
    if fp8:
        for c in range(0, nch, 2):
            nc.tensor.matmul(ps, lhsT=w_ap(c, 2), rhs=x_ap(c, 2),
                             start=(c == 0), stop=(c == nch - 2), perf_mode=DR)
    else:
        for c in range(nch):
            nc.tensor.matmul(ps, lhsT=w_ap(c, 1), rhs=x_ap(c, 1),
                             start=(c == 0), stop=(c == nch - 1))


def build_program(apply_g1=False, apply_g2=False):
    nc = bass.Bass()
    xdt = FP8 if FP8_PROJ else BF16
    mdt = FP8 if FP8_MLP else BF16
    sc_exp = 0.125 / (WS * WS if FP8_PROJ else 1.0)
    aug = WS if FP8_PROJ else 1.0

    xb = nc.declare_dram_parameter("xb", [T, D], F32, isOutput=False)
    xq = nc.declare_dram_parameter("xq", [TQ, D], F32, isOutput=False)
    wq = nc.declare_dram_parameter("wq", [8, P, NCH, P], xdt, isOutput=False)
    wk = nc.declare_dram_parameter("wk", [8, P, NCH, P], xdt, isOutput=False)
    wv = nc.declare_dram_parameter("wv", [8, P, NCH, P], xdt, isOutput=False)
    w1t = nc.declare_dram_parameter("w1t", [32, P, NCH, P], mdt, isOutput=False)
    w2t = nc.declare_dram_parameter("w2t", [8, P, 32, P], mdt, isOutput=False)
    b1t = nc.declare_dram_parameter("b1t", [P, 32], F32, isOutput=False)
    b2t = nc.declare_dram_parameter("b2t", [P, 8], F32, isOutput=False)
    maskt = nc.declare_dram_parameter("maskt", [P, 64], F32, isOutput=False)
    gb = {}
    if apply_g1:
        gb["g1"] = nc.declare_dram_parameter("g1v", [D], F32, isOutput=False)
        gb["be1"] = nc.declare_dram_parameter("be1v", [D], F32, isOutput=False)
    if apply_g2:
        gb["g2"] = nc.declare_dram_parameter("g2v", [D], F32, isOutput=False)
        gb["be2"] = nc.declare_dram_parameter("be2v", [D], F32, isOutput=False)
    out_d = nc.declare_dram_parameter("out", [TQ, D], F32, isOutput=True)

    Exp = mybir.ActivationFunctionType.Exp
    Relu = mybir.ActivationFunctionType.Relu

    with tile.TileContext(nc) as tc:
        with tc.tile_pool(name="consts", bufs=1) as consts, \
             tc.tile_pool(name="big", bufs=1) as big:
            id_f32 = consts.tile([P, P], F32)
            make_identity(nc, id_f32)
            id_bf = consts.tile([P, P], BF16)
            make_identity(nc, id_bf)
            eps_sb = consts.tile([P, 1], F32)
            nc.vector.memset(eps_sb, EPS)
            mask_sb = consts.tile([P, 64], F32)
            nc.sync.dma_start(out=mask_sb, in_=maskt[:, :])
            b1_sb = consts.tile([P, 32], F32)
            nc.sync.dma_start(out=b1_sb, in_=b1t[:, :])
            b2_sb = consts.tile([P, 8], F32)
            nc.sync.dma_start(out=b2_sb, in_=b2t[:, :])
            inv_sb = consts.tile([P, 1], F32)
            nc.vector.memset(inv_sb, 1.0 / (MS * MS))

            def bcast(name):
                t = consts.tile([P, D], F32, tag=f"bc_{name}")
                src = gb[name]
                ap = bass.AP(tensor=src.tensor if hasattr(src, "tensor") else src[:].tensor,
                             offset=src[:].offset, ap=[[0, P]] + list(src[:].ap))
                nc.sync.dma_start(out=t, in_=ap)
                return t

            g1_t = bcast("g1") if apply_g1 else None
            be1_t = bcast("be1") if apply_g1 else None
            g2_t = bcast("g2") if apply_g2 else None
            be2_t = bcast("be2") if apply_g2 else None

            # ---- resident tensors ----
            XT = big.tile([P, NCH, T], xdt)            # LN1(xb)^T
            XQT = big.tile([P, NCH, TQ], xdt)          # LN1(xq)^T
            ACC = big.tile([P, NQ, D], BF16)           # attention out accumulator

            def layernorm_rows(lnp, src_ap, gt, bt):
                """LN over rows of src_ap [128, D] f32 -> xn [128, D] f32."""
                stats = lnp.tile([P, 2, 6], F32, tag="stats")
                for s in range(2):
                    nc.vector.bn_stats(out=stats[:, s, :], in_=src_ap[:, s * 512:(s + 1) * 512])
                mv = lnp.tile([P, 2], F32, tag="mv")
                nc.vector.bn_aggr(out=mv, in_=stats)
                rstd = lnp.tile([P, 1], F32, tag="rstd")
                nc.scalar.activation(out=rstd, in_=mv[:, 1:2],
                                     func=mybir.ActivationFunctionType.Sqrt,
                                     bias=eps_sb, scale=1.0)
                nc.vector.reciprocal(out=rstd, in_=rstd)
                xn = lnp.tile([P, D], F32, tag="xn")
                nc.vector.tensor_scalar(out=xn, in0=src_ap, scalar1=mv[:, 0:1],
                                        scalar2=rstd, op0=mybir.AluOpType.subtract,
                                        op1=mybir.AluOpType.mult)
                if gt is not None:
                    nc.vector.tensor_mul(xn, xn, gt)
                if bt is not None:
                    nc.vector.tensor_add(xn, xn, bt)
                return xn

            def transpose_to(pst, xn, dst3, col, dtype, tag="lntr"):
                """xn [128, D] -> transposed bf16/fp8 into dst3[:, c, col:col+128]."""
                for half in range(2):
                    ps = pst.tile([P, 4, P], F32, tag=tag)
                    for cc in range(4):
                        c = half * 4 + cc
                        nc.tensor.matmul(ps[:, cc, :], lhsT=xn[:, c * P:(c + 1) * P],
                                         rhs=id_f32, is_transpose=True,
                                         start=(cc == 0), stop=(cc == 3),
                                         skip_group_check=True)
                    nc.vector.tensor_copy(dst3[:, half * 4:(half + 1) * 4, col:col + P], ps)

            # ---- Phases 1+2: LN1, then per pair: attention braided with the
            # next pair's QKV projections (dense chains keep the PE clock
            # gate warm through the attention's gappier stream) ----
            mid_cm = tc.tile_pool(name="mid", bufs=1)
            mid = mid_cm.__enter__()
            xv = mid.tile([P, NQ, D], F32)             # residual stream, my tokens
            with tc.tile_pool(name="wpp", bufs=2) as wpp, \
                 tc.tile_pool(name="kqv", bufs=2) as kqv, \
                 tc.tile_pool(name="lnsrc", bufs=3) as lnsrc, \
                 tc.tile_pool(name="lnp", bufs=3) as lnp, \
                 tc.tile_pool(name="scr", bufs=3) as scr, \
                 tc.tile_pool(name="ptp", bufs=3) as ptp, \
                 tc.tile_pool(name="scr2", bufs=2) as scr2, \
                 tc.tile_pool(name="scr3", bufs=4) as scr3, \
                 tc.tile_pool(name="mmp", bufs=2, space="PSUM") as mmp, \
                 tc.tile_pool(name="stp", bufs=2, space="PSUM") as stp, \
                 tc.tile_pool(name="otp", bufs=1, space="PSUM") as otp, \
                 tc.tile_pool(name="trp", bufs=2, space="PSUM") as trp:

                def start_pair(p):
                    """DMA pair p's weights, allocate its K^T/Q^T/V tiles, and
                    return emit-closures for its projection chains."""
                    wqp = wpp.tile([P, NCH, P], xdt, tag="wq")
                    nc.sync.dma_start(out=wqp, in_=wq[p])
                    wkp = wpp.tile([P, NCH, P], xdt, tag="wk")
                    nc.sync.dma_start(out=wkp, in_=wk[p])
                    wvp = wpp.tile([P, NCH, P], xdt, tag="wv")
                    nc.sync.dma_start(out=wvp, in_=wv[p])
                    kt = kqv.tile([P, T], BF16, tag="kt")
                    qt = kqv.tile([P, TQ], BF16, tag="qt")
                    va = kqv.tile([P, 2, NB, 65], BF16, tag="va")
                    nc.vector.memset(va[:, :, :, 64:65], aug)
                    vts = {}

                    def kchain(tg):
                        ps = mmp.tile([P, 512], F32, tag="mm")
                        _mm_chain(nc, ps,
                                  lambda c, k: wkp[:, c, :] if k == 1 else wkp[:, c:c + k, :],
                                  lambda c, k: XT[:, c, tg * 512:(tg + 1) * 512] if k == 1
                                  else XT[:, c:c + k, tg * 512:(tg + 1) * 512],
                                  NCH, FP8_PROJ)
                        nc.vector.tensor_copy(kt[:, tg * 512:(tg + 1) * 512], ps)

                    def vchain(tg):
                        ps = mmp.tile([P, 512], F32, tag="mm")
                        _mm_chain(nc, ps,
                                  lambda c, k: wvp[:, c, :] if k == 1 else wvp[:, c:c + k, :],
                                  lambda c, k: XT[:, c, tg * 512:(tg + 1) * 512] if k == 1
                                  else XT[:, c:c + k, tg * 512:(tg + 1) * 512],
                                  NCH, FP8_PROJ)
                        vt = scr.tile([P, 512], BF16, tag="vt")
                        nc.vector.tensor_copy(vt, ps)
                        vts[tg] = vt

                    def vtrans(tg):
                        vt = vts.pop(tg)
                        for hh in range(2):
                            tps = trp.tile([P, 4, 66], BF16, tag="tr")
                            for s in range(4):
                                nc.tensor.matmul(
                                    tps[:, s, 0:64],
                                    lhsT=vt[hh * 64:(hh + 1) * 64, s * P:(s + 1) * P],
                                    rhs=id_bf[hh * 64:hh * 64 + 64, hh * 64:hh * 64 + 64],
                                    is_transpose=True, start=(s == 0), stop=(s == 3),
                                    skip_group_check=True)
                            nc.vector.tensor_copy(va[:, hh, tg * 4:(tg + 1) * 4, 0:64],
                                                  tps[:, :, 0:64])

                    def qchain(g):
                        ps = mmp.tile([P, 512], F32, tag="mm")
                        _mm_chain(nc, ps,
                                  lambda c, k: wqp[:, c, :] if k == 1 else wqp[:, c:c + k, :],
                                  lambda c, k: XQT[:, c, g * 512:(g + 1) * 512] if k == 1
                                  else XQT[:, c:c + k, g * 512:(g + 1) * 512],
                                  NCH, FP8_PROJ)
                        nc.vector.tensor_copy(qt[:, g * 512:(g + 1) * 512], ps)

                    def em(f, *a):
                        return lambda: f(*a)
                    ems = [em(kchain, 0), em(vchain, 0), em(kchain, 1), em(vtrans, 0),
                           em(vchain, 1), em(kchain, 2), em(vtrans, 1), em(vchain, 2),
                           em(kchain, 3), em(vtrans, 2), em(vchain, 3), em(qchain, 0),
                           em(vtrans, 3), em(qchain, 1)]
                    return (kt, qt, va), ems

                def attention_pair(p, tiles, braid):
                    kt, qt, va = tiles
                    bi = [0]
                    step = [0]
                    nsteps = 2 * NB

                    def pop():
                        # pace the braid evenly over the pair's j-steps; once
                        # dry, keep the PE clock gate warm with a dummy matmul
                        step[0] += 1
                        want = (len(braid) * step[0]) // nsteps
                        while bi[0] < min(want, len(braid)):
                            braid[bi[0]]()
                            bi[0] += 1
                        if bi[0] >= len(braid) and step[0] % 2 == 0:
                            dst = stp.tile([P, 512], F32, tag="st")
                            nc.tensor.matmul(dst, lhsT=id_bf,
                                             rhs=kt[:, 0:512],
                                             start=True, stop=True)

                    for hh in range(2):
                        h = 2 * p + hh
                        hs = slice(hh * 64, (hh + 1) * 64)
                        OT = otp.tile([65, TQ], F32, tag="ot")
                        ot_sb = scr2.tile([65, TQ], F32, tag="otsb")

                        def drain(lo):
                            nc.vector.tensor_copy(ot_sb[:, lo:lo + 512], OT[:, lo:lo + 512])
                            for kb in range(lo // P, lo // P + 4):
                                o_ps = trp.tile([P, 65], F32, tag="tr")
                                nc.tensor.transpose(o_ps, ot_sb[:, kb * P:(kb + 1) * P],
                                                    id_f32[0:65, 0:65])
                                rd = scr3.tile([P, 1], F32, tag="rd")
                                nc.vector.reciprocal(rd, o_ps[:, 64:65])
                                nc.vector.tensor_scalar_mul(
                                    ACC[:, kb, h * HD:(h + 1) * HD], o_ps[:, 0:64], rd)

                        def emit_av(pt_j, j):
                            # split at the PSUM bank boundary (col 512)
                            base = 64 * j
                            segs = (([(base, 512)] if base < 512 else [])
                                    + [(max(base, 512), TQ)])
                            for (s0, s1) in segs:
                                nc.tensor.matmul(OT[:, s0:s1], lhsT=va[:, hh, j, :],
                                                 rhs=pt_j[:, s0 - base:s1 - base],
                                                 start=(j == 0),
                                                 stop=(j == 7 and s1 == 512) or (j == 15),
                                                 skip_group_check=True)
                            if j == 7:
                                drain(0)

                        pending = None  # scores of j+1 are emitted before AV of j
                        for j in range(NB):
                            slen = TQ - 64 * j
                            base = 64 * j
                            pt_j = ptp.tile([P, TQ], BF16, tag="pt")
                            pos = 0
                            while pos < slen:
                                w = min(512, slen - pos)
                                st = stp.tile([P, 512], F32, tag="st")
                                nc.tensor.matmul(st[:, 0:w],
                                                 lhsT=kt[hs, j * P:(j + 1) * P],
                                                 rhs=qt[hs, base + pos:base + pos + w],
                                                 start=True, stop=True)
                                if pos == 0:
                                    nc.vector.tensor_add(st[:, 0:64], st[:, 0:64], mask_sb)
                                nc.scalar.activation(out=pt_j[:, pos:pos + w],
                                                     in_=st[:, 0:w], func=Exp, scale=sc_exp)
                                pos += w
                            if pending is not None:
                                emit_av(*pending)
                            pending = (pt_j, j)
                            pop()
                        emit_av(*pending)
                        drain(512)
                    while bi[0] < len(braid):
                        braid[bi[0]]()
                        bi[0] += 1

                # LN1 for xb and xq, braided with pair 0's projections
                tiles0, ems0 = start_pair(0)
                e0 = 0
                for tg in range(4):
                    for bi2 in range(4):
                        blk = tg * 4 + bi2
                        x_t = lnsrc.tile([P, D], F32, tag="xsrc")
                        nc.sync.dma_start(out=x_t, in_=xb[blk * P:(blk + 1) * P, :])
                        xn = layernorm_rows(lnp, x_t, g1_t, be1_t)
                        transpose_to(mmp, xn, XT, tg * 512 + bi2 * P, xdt, tag="mm")
                    for qi in range(2):
                        kb = tg * 2 + qi
                        xq_t = lnsrc.tile([P, D], F32, tag="xsrc")
                        nc.sync.dma_start(out=xq_t, in_=xq[kb * P:(kb + 1) * P, :])
                        xn = layernorm_rows(lnp, xq_t, g1_t, be1_t)
                        transpose_to(mmp, xn, XQT, kb * P, xdt, tag="mm")
                    # pair-0 chains for windows that are now complete
                    while e0 < [2, 5, 8, 14][tg]:
                        ems0[e0]()
                        e0 += 1
                for kb in range(NQ):
                    nc.sync.dma_start(out=xv[:, kb, :], in_=xq[kb * P:(kb + 1) * P, :])

                tiles, ems = tiles0, ems0
                for p in range(8):
                    nxt = start_pair(p + 1) if p < 7 else (None, [])
                    attention_pair(p, tiles, nxt[1])
                    tiles = nxt[0]
            # ---- Phase 3: LN2 + MLP + residual, per 512-token group ----
            with tc.tile_pool(name="x2tp", bufs=2) as x2tp, \
                 tc.tile_pool(name="h1p", bufs=1) as h1p, \
                 tc.tile_pool(name="w1s", bufs=3) as w1s, \
                 tc.tile_pool(name="w2s", bufs=2) as w2s, \
                 tc.tile_pool(name="lnp2", bufs=2) as lnp2, \
                 tc.tile_pool(name="scr4", bufs=2) as scr4, \
                 tc.tile_pool(name="mmd", bufs=3, space="PSUM") as mmd, \
                 tc.tile_pool(name="trd", bufs=2, space="PSUM") as trd:
                for g in range(2):
                    X2T = x2tp.tile([P, NCH, 512], mdt, tag="x2t")
                    for s in range(4):
                        kb = g * 4 + s
                        nc.vector.tensor_add(xv[:, kb, :], xv[:, kb, :], ACC[:, kb, :])
                        xn = layernorm_rows(lnp2, xv[:, kb, :], g2_t, be2_t)
                        transpose_to(trd, xn, X2T, s * P, mdt)
                    h1 = h1p.tile([P, 32, 512], mdt, tag="h1")
                    for f in range(32):
                        w1f = w1s.tile([P, NCH, P], mdt, tag="w1f")
                        nc.sync.dma_start(out=w1f, in_=w1t[f])
                        ps = mmd.tile([P, 512], F32, tag="mm")
                        _mm_chain(nc, ps,
                                  lambda c, k: w1f[:, c, :] if k == 1 else w1f[:, c:c + k, :],
                                  lambda c, k: X2T[:, c, :] if k == 1 else X2T[:, c:c + k, :],
                                  NCH, FP8_MLP)
                        nc.scalar.activation(out=h1[:, f, :], in_=ps, func=Relu,
                                             bias=b1_sb[:, f:f + 1], scale=1.0)
                    # W2 stage; output transposes lag one dd so the PE never
                    # waits on the DVE bias-add of fsb.
                    def emit_out(fsb, dd):
                        for s in range(4):
                            kb = g * 4 + s
                            tp = trd.tile([P, 4, P], F32, tag="lntr")
                            nc.tensor.matmul(tp[:, 0, :], lhsT=fsb[:, s * P:(s + 1) * P],
                                             rhs=id_f32, is_transpose=True,
                                             start=True, stop=True)
                            nc.vector.tensor_add(xv[:, kb, dd * P:(dd + 1) * P],
                                                 xv[:, kb, dd * P:(dd + 1) * P],
                                                 tp[:, 0, :])

                    pending_o = None
                    for dd in range(8):
                        w2d = w2s.tile([P, 32, P], mdt, tag="w2d")
                        nc.sync.dma_start(out=w2d, in_=w2t[dd])
                        ps = mmd.tile([P, 512], F32, tag="mm")
                        _mm_chain(nc, ps,
                                  lambda c, k: w2d[:, c, :] if k == 1 else w2d[:, c:c + k, :],
                                  lambda c, k: h1[:, c, :] if k == 1 else h1[:, c:c + k, :],
                                  32, FP8_MLP)
                        fsb = scr4.tile([P, 512], F32, tag="fsb")
                        if FP8_MLP:
                            nc.vector.tensor_scalar(out=fsb, in0=ps, scalar1=inv_sb,
                                                    scalar2=b2_sb[:, dd:dd + 1],
                                                    op0=mybir.AluOpType.mult,
                                                    op1=mybir.AluOpType.add)
                        else:
                            nc.vector.tensor_scalar_add(fsb, ps, b2_sb[:, dd:dd + 1])
                        if pending_o is not None:
                            emit_out(*pending_o)
                        pending_o = (fsb, dd)
                    emit_out(*pending_o)
                    for s in range(4):
                        kb = g * 4 + s
                        nc.sync.dma_start(out=out_d[kb * P:(kb + 1) * P, :],
                                          in_=xv[:, kb, :])
            mid_cm.__exit__(None, None, None)

    _split_drain_waits(nc)
    return nc


def _split_drain_waits(nc):
    """This walrus build gives every instruction a single hardware wait slot
    (one EVENTS struct per 64B instruction). Tile emits multi-wait
    instructions; move the excess waits onto single-wait NoOps inserted just
    before, on the same engine — identical semantics in program order."""
    for fn in nc.m.functions:
        for blk in fn.blocks:
            insts = blk.instructions
            i = 0
            while i < len(insts):
                inst = insts[i]
                si = inst.sync_info
                if si is not None and len(si.on_wait) > 1:
                    waits = list(si.on_wait)
                    inst.sync_info = mybir.SyncInfo(on_wait=[waits[-1]],
                                                    on_update=list(si.on_update))
                    for w in waits[:-1]:
                        nop = mybir.InstNoOp(name=nc.get_next_instruction_name(),
                                             ins=[], outs=[])
                        nop.engine = inst.engine
                        nop.sync_info = mybir.SyncInfo(on_wait=[w], on_update=[])
                        nc.register_instruction(nop, overwrite=True)
                        insts.insert(i, nop)
                        i += 1
                i += 1


def _prep_inputs(inputs, Wq, Wk, Wv, W1, b1, W2, b2, g1, be1, g2, be2,
                 apply_g1, apply_g2):
    bf = ml_dtypes.bfloat16
    f8 = ml_dtypes.float8_e4m3
    f32 = np.float32
    xdt = f8 if FP8_PROJ else bf
    mdt = f8 if FP8_MLP else bf
    pws = WS if FP8_PROJ else 1.0
    mws = MS if FP8_MLP else 1.0
    inputs = np.ascontiguousarray(np.asarray(inputs, f32))
    wq_f = np.asarray(Wq, f32).transpose(1, 0, 2).reshape(D, D) * pws
    wk_f = np.asarray(Wk, f32).transpose(1, 0, 2).reshape(D, D) * pws
    wv_f = np.asarray(Wv, f32).transpose(1, 0, 2).reshape(D, D) * pws

    def pair_tiles(w, dt):  # [D, D] -> [8, 128, 8, 128] (pair, p, chunk, col)
        if dt is f8:
            w = np.clip(w, -240.0, 240.0)
        return np.ascontiguousarray(
            w.reshape(NCH, P, 8, P).transpose(2, 1, 0, 3).astype(dt))

    wq_t = pair_tiles(wq_f, xdt)
    wk_t = pair_tiles(wk_f, xdt)
    wv_t = pair_tiles(wv_f, xdt)
    w1_t = np.ascontiguousarray(
        (np.asarray(W1, f32) * mws).reshape(NCH, P, 32, P).transpose(2, 1, 0, 3).astype(mdt))
    w2_t = np.ascontiguousarray(
        (np.asarray(W2, f32) * mws).reshape(32, P, 8, P).transpose(2, 1, 0, 3).astype(mdt))
    b1_t = np.ascontiguousarray((np.asarray(b1, f32) * mws).reshape(32, P).T)
    b2_t = np.ascontiguousarray(np.asarray(b2, f32).reshape(8, P).T)

    in_maps = []
    for c in range(8):
        b, o = divmod(c, 2)
        xb_c = inputs[b]
        xq_c = np.ascontiguousarray(xb_c[o::2, :])
        cc, kk = np.meshgrid(np.arange(P), np.arange(64), indexing="ij")
        mask = np.where(cc <= 2 * kk + o, 0.0, NEG * pws * pws).astype(f32)
        m = {"xb": xb_c, "xq": xq_c, "wq": wq_t, "wk": wk_t, "wv": wv_t,
             "w1t": w1_t, "w2t": w2_t, "b1t": b1_t, "b2t": b2_t, "maskt": mask}
        if apply_g1:
            m["g1v"] = np.asarray(g1, f32)
            m["be1v"] = np.asarray(be1, f32)
        if apply_g2:
            m["g2v"] = np.asarray(g2, f32)
            m["be2v"] = np.asarray(be2, f32)
        in_maps.append(m)
    return in_maps


def _run(inputs, Wq, Wk, Wv, W1, b1, W2, b2, g1, be1, g2, be2, **spmd_kwargs):
    apply_g1 = not (np.all(np.asarray(g1) == 1.0) and np.all(np.asarray(be1) == 0.0))
    apply_g2 = not (np.all(np.asarray(g2) == 1.0) and np.all(np.asarray(be2) == 0.0))
    nc = build_program(apply_g1, apply_g2)
    in_maps = _prep_inputs(inputs, Wq, Wk, Wv, W1, b1, W2, b2, g1, be1, g2, be2,
                           apply_g1, apply_g2)
    res = run_bass_kernel_spmd(nc, in_maps, list(range(8)), **spmd_kwargs)
    out = np.empty((B, T, D), np.float32)
    for c in range(8):
        b, o = divmod(c, 2)
        out[b, o::2, :] = res.results[c]["out"]
    return out, res


def kernel(inputs, Wq, Wk, Wv, W1, b1, W2, b2, g1, be1, g2, be2):
    out, _ = _run(inputs, Wq, Wk, Wv, W1, b1, W2, b2, g1, be1, g2, be2)
    return out
